# revision 26
# baseline (speedup 1.0000x reference)
"""BasisVQ forward on 8 TRN2 NeuronCores (Bass/Tile).

Math restructuring (exact in real arithmetic):
  z = x @ W^T + b                                  [8192, 2700]
  S = z @ E^T  = x @ Me^T + (b @ E^T),  Me = E @ W [1024, 256]
  dist = |z|^2 - 2 S + |e|^2
  argmin_c dist == argmin_c G,     G = -2 x Me^T + (e2 - 2 be)   (drops |z|^2 row-const)
  softmax(-dist) == exp(m - G)/sum (row-const cancels),  m = row min of G
  q_st  = z + sg(e_i - z) = e_i  (gather)
  vq_loss = mean(min dist) = (sum_r |z_r|^2 + m_r) / (8192*2700)
  |z_r|^2 = x Gram x^T + 2 x.wb + |b|^2,  Gram = W^T W, wb = W^T b

Sharding: data-parallel over B (1024 rows/core). The codebook projection
Me = E @ W is sharded over codes (128 codes/core) and AllGathered (1 MB);
Gram stays replicated (cheap) and T2/zsq run during the collective.
"""

import numpy as np

B, K, D = 128, 64, 256
BD, NC = 2700, 1024
CORES = 8
R = (B * K) // CORES          # rows per core = 1024
RT = R // 128                 # row tiles per core = 8
TB = 22                       # bd tiles (2816 = 22*128)
BDP = TB * 128                # padded basis dim
CS = NC // CORES              # codes per core = 128

_cache = {}


def _build(reps=1):
    import concourse.mybir as mybir
    import concourse.tile as tile
    import concourse.bass as bass
    from concourse import bacc

    f32 = mybir.dt.float32
    bf16 = mybir.dt.bfloat16
    u32 = mybir.dt.uint32

    nc = bacc.Bacc("TRN2", num_devices=CORES)

    xT_d = nc.dram_tensor("xT", [128, 2 * R], f32, kind="ExternalInput")
    xR_d = nc.dram_tensor("xR", [128, RT * D], f32, kind="ExternalInput")
    wb_d = nc.dram_tensor("wb", [128, TB * (D + 1)], f32, kind="ExternalInput")
    ets_d = nc.dram_tensor("ets", [128, TB * CS], f32, kind="ExternalInput")
    cv_d = nc.dram_tensor("cv", [1, NC], f32, kind="ExternalInput")
    emb_d = nc.dram_tensor("emb", [NC, BD], f32, kind="ExternalInput")

    q_d = nc.dram_tensor("q", [R, BD], f32, kind="ExternalOutput")
    idx_d = nc.dram_tensor("idx", [RT, 128], u32, kind="ExternalOutput")
    m_d = nc.dram_tensor("m", [RT, 128], f32, kind="ExternalOutput")
    zsq_d = nc.dram_tensor("zsq", [RT, 128], f32, kind="ExternalOutput")
    xwb_d = nc.dram_tensor("xwb", [RT, 128], f32, kind="ExternalOutput")
    pr_d = nc.dram_tensor("pr", [1, NC], f32, kind="ExternalOutput")

    with tile.TileContext(nc) as tc:
        with (
            tc.tile_pool(name="const", bufs=1) as const,
            tc.tile_pool(name="work", bufs=4) as work,
            tc.tile_pool(name="small", bufs=8) as small,
            tc.tile_pool(name="gat", bufs=3) as gat,
        ):
            # ---- constant loads ----
            w_sb = const.tile([128, TB, D + 1], f32)
            for t0, t1 in ((0, 2), (2, 8), (8, 15), (15, TB)):
                nc.scalar.dma_start(
                    w_sb[:, t0:t1, :],
                    wb_d[:, t0 * (D + 1):t1 * (D + 1)])
            xT_sb = const.tile([128, 2, R], f32)
            nc.scalar.dma_start(xT_sb[:], xT_d[:])
            xR_sb = const.tile([128, RT, D], f32)
            nc.scalar.dma_start(xR_sb[:], xR_d[:])
            ones_bf = const.tile([128, 1], bf16)
            nc.vector.memset(ones_bf[:], 1.0)
            cv_sb = const.tile([128, NC], f32)
            cv_bc = bass.AP(tensor=cv_d.ap().tensor, offset=0,
                            ap=[[0, 128], [1, NC]])
            nc.scalar.dma_start(cv_sb[:], cv_bc)

            for _rep in range(reps):
                cc_in = nc.dram_tensor(f"cc_in{_rep}", [2 * 128, CS], f32,
                                       kind="Internal")
                cc_out = nc.dram_tensor(f"cc_out{_rep}", [CORES * 2 * 128, CS], f32,
                                        kind="Internal", addr_space="Shared")
                me_sb = const.tile([128, 2, NC], f32, name=f"me_sb{_rep}")
                gram_sb = const.tile([128, 2, D + 1], f32, name=f"gram_sb{_rep}")

                # ---- stage B: local Me2 slice + full Gram|wb ----
                with tc.tile_pool(name="psB", bufs=1, space="PSUM") as psB:
                    mep = [psB.tile([128, CS], f32, tag=f"mep{d}",
                                    name=f"mep{d}") for d in range(2)]
                    gr_ps = [psB.tile([128, D + 1], f32, tag=f"gr{d}",
                                      name=f"gr_ps{d}") for d in range(2)]
                    with tc.high_priority():
                        et_sb = const.tile([128, TB, CS], f32, name="et_sb")
                        for t0, t1 in ((0, 2), (2, 8), (8, 15), (15, TB)):
                            nc.sync.dma_start(et_sb[:, t0:t1, :],
                                              ets_d[:, t0 * CS:t1 * CS])
                        for t in range(TB):
                            st, sp = (t == 0), (t == TB - 1)
                            for d2 in range(2):
                                lhsT = w_sb[:, t, d2 * 128:(d2 + 1) * 128]
                                nc.tensor.matmul(mep[d2][:], lhsT,
                                                 et_sb[:, t, :],
                                                 start=st, stop=sp)
                        me_loc = work.tile([128, 2, CS], f32, tag="me_loc")
                        for d2 in range(2):
                            nc.vector.tensor_copy(me_loc[:, d2, :], mep[d2][:])
                        nc.sync.dma_start(
                            cc_in.ap().rearrange("(dt p) c -> p dt c", p=128),
                            me_loc[:])
                        nc.gpsimd.collective_compute(
                            "AllGather", mybir.AluOpType.bypass,
                            replica_groups=[list(range(CORES))],
                            ins=[cc_in[:]], outs=[cc_out[:]])

                    with tc.tile_wait_until(0.034):
                        for t in range(TB):
                            st, sp = (t == 0), (t == TB - 1)
                            for d2 in range(2):
                                lhsT = w_sb[:, t, d2 * 128:(d2 + 1) * 128]
                                nc.tensor.matmul(gr_ps[d2][:], lhsT,
                                                 w_sb[:, t, :],
                                                 start=st, stop=sp)
                        for d2 in range(2):
                            nc.scalar.copy(gram_sb[:, d2, :], gr_ps[d2][:])

                # ---- T2 / zsq / xwb (overlaps the AllGather) ----
                with tc.tile_pool(name="psT", bufs=2, space="PSUM") as psT:
                    for rt in range(RT):
                        t2_ps = psT.tile([128, D + 1], f32, tag="t2", name="t2_ps")
                        for d2 in range(2):
                            lhsT = xT_sb[:, d2, rt * 128:(rt + 1) * 128]
                            nc.tensor.matmul(t2_ps[:], lhsT, gram_sb[:, d2, :],
                                             start=(d2 == 0), stop=(d2 == 1))
                        prod = work.tile([128, D], f32, tag="prod")
                        zsq1 = small.tile([128, 1], f32, tag="zsq1")
                        nc.vector.tensor_mul(prod[:], xR_sb[:, rt, :], t2_ps[:, 0:D])
                        nc.vector.tensor_reduce(zsq1[:], prod[:],
                                                axis=mybir.AxisListType.X,
                                                op=mybir.AluOpType.add)
                        xwb1 = small.tile([128, 1], f32, tag="xwb1")
                        nc.vector.tensor_copy(xwb1[:], t2_ps[:, D:D + 1])
                        nc.sync.dma_start(zsq_d[rt, :], zsq1[:])
                        nc.sync.dma_start(xwb_d[rt, :], xwb1[:])

                # ---- gathered Me (code c' = g*128 + j at position g*128+j) ----
                cc_out_r = cc_out.ap().rearrange("(g dt p) c -> p dt g c",
                                                 p=128, dt=2)
                for dt in range(2):
                    nc.scalar.dma_start(
                        me_sb[:, dt, :].rearrange("p (g c) -> p g c", g=CORES),
                        cc_out_r[:, dt, :, :])

                # ---- stage C: per row-tile ----
                with tc.tile_pool(name="psC", bufs=1, space="PSUM") as psC:
                    pr_ps = [psC.tile([1, 512], f32, tag=f"pr{c}", name=f"pr_ps{c}")
                             for c in range(2)]
                    for rt in range(RT):
                        G = work.tile([128, NC], f32, tag="G", bufs=4,
                                      name=f"G{rt}")
                        g_ps = [psC.tile([128, 512], f32, tag=f"g{h}",
                                         name=f"g_ps{h}", bufs=2)
                                for h in range(2)]
                        for d2 in range(2):
                            lhsT = xT_sb[:, d2, rt * 128:(rt + 1) * 128]
                            for h in range(2):
                                nc.tensor.matmul(
                                    g_ps[h][:], lhsT,
                                    me_sb[:, d2, h * 512:(h + 1) * 512],
                                    start=(d2 == 0), stop=(d2 == 1))
                        for h in range(2):
                            nc.vector.tensor_add(
                                G[:, h * 512:(h + 1) * 512], g_ps[h][:],
                                cv_sb[:, h * 512:(h + 1) * 512])

                        m1 = small.tile([128, 1], f32, tag="m1")
                        nc.vector.tensor_reduce(m1[:], G[:],
                                                axis=mybir.AxisListType.X,
                                                op=mybir.AluOpType.min)
                        idx8 = small.tile([128, 8], u32, tag="idx8")
                        nc.vector.max_index(idx8[:], m1[:].to_broadcast([128, 8]),
                                            G[:])
                        eg = gat.tile([128, BD], f32, tag="eg")
                        nc.gpsimd.indirect_dma_start(
                            out=eg[:], out_offset=None, in_=emb_d[:],
                            in_offset=bass.IndirectOffsetOnAxis(ap=idx8[:, 0:1],
                                                                axis=0))
                        qeng = nc.sync if rt % 2 == 0 else nc.scalar
                        qeng.dma_start(q_d[rt * 128:(rt + 1) * 128, :], eg[:])
                        nc.sync.dma_start(idx_d[rt, :], idx8[:, 0:1])
                        nc.sync.dma_start(m_d[rt, :], m1[:])
                        u = work.tile([128, NC], f32, tag="u")
                        s1 = small.tile([128, 1], f32, tag="s1")
                        nc.scalar.activation(u[:], G[:],
                                             mybir.ActivationFunctionType.Exp,
                                             bias=m1[:], scale=-1.0, accum_out=s1[:])
                        rinv = small.tile([128, 1], f32, tag="rinv")
                        nc.vector.reciprocal(rinv[:], s1[:])
                        pbf = work.tile([128, NC], bf16, tag="pbf")
                        nc.scalar.mul(pbf[:], u[:], rinv[:])
                        for ch in range(2):
                            nc.tensor.matmul(pr_ps[ch][:], ones_bf[:],
                                             pbf[:, ch * 512:(ch + 1) * 512],
                                             start=(rt == 0), stop=(rt == RT - 1))


                    pr_sb = small.tile([1, NC], f32, tag="prsb")
                    for ch in range(2):
                        nc.vector.tensor_copy(pr_sb[:, ch * 512:(ch + 1) * 512],
                                              pr_ps[ch][:])
                    nc.sync.dma_start(pr_d[:], pr_sb[:])


    nc.compile()
    return nc


def _get_nc(reps=1):
    key = f"nc{reps}"
    if key not in _cache:
        _cache[key] = _build(reps)
    return _cache[key]


def _code_perm():
    # single AllGather: device code order equals true code order
    return np.arange(NC, dtype=np.int64)


def _prep_shared(W_proj, b_proj, embed):
    Wb = np.zeros((BDP, D + 1), np.float32)
    Wb[:BD, :D] = W_proj
    Wb[:BD, D] = b_proj
    Wb = np.ascontiguousarray(
        Wb.reshape(TB, 128, D + 1).transpose(1, 0, 2).reshape(128, TB * (D + 1)))
    e2 = (embed.astype(np.float64) ** 2).sum(1)
    be = embed.astype(np.float64) @ b_proj.astype(np.float64)
    cvec = (e2 - 2.0 * be).astype(np.float32)
    perm = _code_perm()
    cv = np.ascontiguousarray(cvec[perm].reshape(1, NC))
    return Wb, cv, perm


def kernel(slot_features, W_proj, b_proj, embed, _want_results=False):
    from concourse.bass_utils import run_bass_kernel_spmd

    slot_features = np.ascontiguousarray(slot_features, np.float32)
    W_proj = np.ascontiguousarray(W_proj, np.float32)
    b_proj = np.ascontiguousarray(b_proj, np.float32)
    embed = np.ascontiguousarray(embed, np.float32)

    x = slot_features.reshape(B * K, D)
    Wb, cv, perm = _prep_shared(W_proj, b_proj, embed)
    emb_perm = np.ascontiguousarray(embed[perm])

    in_maps = []
    for c in range(CORES):
        xs = x[c * R:(c + 1) * R]
        ets = np.zeros((BDP, CS), np.float32)
        ets[:BD] = (-2.0) * embed[c * CS:(c + 1) * CS, :].T
        ets = np.ascontiguousarray(
            ets.reshape(TB, 128, CS).transpose(1, 0, 2).reshape(128, TB * CS))
        xT = np.ascontiguousarray(
            xs.T.reshape(2, 128, R).transpose(1, 0, 2).reshape(128, 2 * R))
        xR = np.ascontiguousarray(
            xs.reshape(RT, 128, D).transpose(1, 0, 2).reshape(128, RT * D))
        in_maps.append({
            "xT": xT, "xR": xR,
            "wb": Wb, "ets": ets, "cv": cv, "emb": emb_perm,
        })

    nc = _get_nc()
    res = run_bass_kernel_spmd(nc, in_maps, core_ids=list(range(CORES)))
    outs = res.results

    q = np.concatenate([r["q"] for r in outs], axis=0).reshape(B, K, BD)
    idx = np.concatenate([r["idx"].reshape(-1) for r in outs]).astype(np.int64)
    indices = perm[idx].astype(np.int32).reshape(B, K)

    b2 = float((b_proj.astype(np.float64) ** 2).sum())
    tot = 0.0
    for r in outs:
        tot += (r["zsq"].astype(np.float64).sum()
                + 2.0 * r["xwb"].astype(np.float64).sum()
                + r["m"].astype(np.float64).sum())
    tot += (B * K) * b2
    vq_loss = np.float32(tot / float(B * K * BD))

    pr_tot = np.zeros(NC, np.float64)
    for r in outs:
        pr_tot += r["pr"].reshape(-1).astype(np.float64)
    avg_p = np.empty(NC, np.float64)
    avg_p[perm] = pr_tot / float(B * K)
    entropy = np.float32(-(avg_p * np.log(avg_p + 1e-8)).sum())

    if _want_results:
        return (q, indices, vq_loss, entropy), res
    return q, indices, vq_loss, entropy
